# revision 13
# baseline (speedup 1.0000x reference)
"""nn_MultiHeadAttention_59253368815813 on 8 TRN2 NeuronCores.

The reference module is bug-faithful to its original nn.Module in two ways
that together collapse the computation:

  1. ``o = jnp.einsum('bhtl,bthd->bhtd', A, v)`` indexes ``v`` by the QUERY
     position ``t``, not the key position ``l``. ``l`` therefore only sums
     over the softmax weights, which sum to exactly 1 per row:
     ``o[b,h,t,d] == v[b,t,h,d]``. Q, K, the mask and the softmax never
     influence the output (verified vs the reference to 4e-7 rel).
  2. ``o.reshape(b, T, d)`` with no transpose scrambles (head, token) so the
     reshaped activation row tj = 128*h + s is the concatenation over
     m=0..15 of v[b, 16*s+m, h, :].

So the exact computation is  out = scramble(x @ Wv) @ Wo.T,  and the
scramble makes output rows depend on one head only. Sharding: core c owns
heads {2c, 2c+1}, i.e. Wv columns [128c, 128c+128) and output rows
[256c, 256c+256) of each batch; the host concatenates the row slabs.
No cross-core reduction needed.

Per core (fp32r matmuls, fp32 PSUM):
  vT[128ch, b*T+t] = Wv_slice^T @ x^T          (x^T fed from host)
  out[128h + s, n] = sum_m sum_di vT[64h+di, 16s+m] * Wo.T[64m+di, n]
    -> 16 accumulating matmuls whose lhsT is a stride-16 slice of vT.
"""

import sys
import types

import numpy as np

_TRN_REPO = "/opt/trn_rl_repo"
if _TRN_REPO not in sys.path:
    sys.path.insert(0, _TRN_REPO)


def _install_ntff_shim():
    """antenv.axon_hooks is absent in this container; provide it so
    BASS_TRACE=1 profiling works. No-op if the real module exists."""
    try:
        import antenv  # noqa: F401
    except ImportError:
        return
    if "antenv.axon_hooks" in sys.modules:
        return
    try:
        import antenv.axon_hooks  # noqa: F401
        return
    except ImportError:
        pass
    m = types.ModuleType("antenv.axon_hooks")
    m._hook = None
    m.set_axon_ntff_profile_hook = lambda h: setattr(m, "_hook", h)
    m.get_axon_ntff_profile_hook = lambda: m._hook
    sys.modules["antenv.axon_hooks"] = m
    try:
        from trn_agent_boot.trn_boot import _ntff_profile_via_ctypes

        hook = _ntff_profile_via_ctypes("/opt/axon/libaxon_pjrt.so")
        if hook is not None:
            m.set_axon_ntff_profile_hook(hook)
    except Exception:
        pass


_install_ntff_shim()

import concourse.mybir as mybir  # noqa: E402
import concourse.tile as tile  # noqa: E402
from concourse import bacc  # noqa: E402
from concourse.bass_utils import run_bass_kernel_spmd  # noqa: E402

F32 = mybir.dt.float32
F32R = mybir.dt.float32r

B = 2
T = 2048
D = 1024
NCORES = 8
GT = B * T          # 4096
NG = GT // 512      # 8 global 512-token chunks
NDCH = D // 128     # 8 contraction chunks for the projection

_CACHED = None
LAST_RESULTS = None


def _build_module():
    nc = bacc.Bacc("TRN2", target_bir_lowering=False, debug=False,
                   num_devices=NCORES)

    xT_d = nc.dram_tensor("xT", [D, GT], F32R, kind="ExternalInput").ap()
    wv_d = nc.dram_tensor("wv", [D, 128], F32R, kind="ExternalInput").ap()
    wo_d = nc.dram_tensor("woT", [64, 16, D], F32R, kind="ExternalInput").ap()
    out_d = nc.dram_tensor("out", [B, 256, D], F32, kind="ExternalOutput").ap()

    with tile.TileContext(nc) as tc:
        _emit(nc, tc, xT_d, wv_d, wo_d, out_d)
    nc.compile()
    return nc


def _emit(nc, tc, xT_d, wv_d, wo_d, out_d):
    from contextlib import ExitStack

    ctx = ExitStack()
    with ctx:
        wpool = ctx.enter_context(tc.tile_pool(name="w", bufs=1))
        xtp = ctx.enter_context(tc.tile_pool(name="xt", bufs=3))
        vtp = ctx.enter_context(tc.tile_pool(name="vt", bufs=1))
        outp = ctx.enter_context(tc.tile_pool(name="outsb", bufs=3))
        ps_p = ctx.enter_context(tc.tile_pool(name="ps_p", bufs=4, space="PSUM"))
        ps_w = ctx.enter_context(tc.tile_pool(name="ps_w", bufs=2, space="PSUM"))

        # weights ride the ACT HWDGE ring; activations the SP ring (parallel)
        wv_sb = wpool.tile([128, NDCH, 128], F32R, tag="wv")
        nc.scalar.dma_start(wv_sb[:],
                            wv_d.rearrange("(ko ki) m -> ki ko m", ki=128))
        wo_sb = wpool.tile([64, 16, D], F32R, tag="wo")
        nc.scalar.dma_start(wo_sb[:], wo_d)

        # vT split by head so both heads' Wo matmuls read base-partition 0
        vt = [vtp.tile([64, GT], F32R, tag=f"vt{h}", name=f"vt{h}")
              for h in range(2)]

        def proj_half(half):
            """v^T for one 2048-token half (= one batch)."""
            pss = [ps_p.tile([128, 512], F32, tag="proj",
                             name=f"psp{half}_{q}") for q in range(4)]
            for dch in range(NDCH):
                xt = xtp.tile([128, 2048], F32R, tag="xt",
                              name=f"xt{half}_{dch}")
                nc.sync.dma_start(
                    xt[:], xT_d[dch * 128:(dch + 1) * 128,
                                half * 2048:(half + 1) * 2048])
                for q in range(4):
                    nc.tensor.matmul(pss[q][:], wv_sb[:, dch, :],
                                     xt[:, q * 512:(q + 1) * 512],
                                     start=(dch == 0), stop=(dch == NDCH - 1))
            for q in range(4):
                for h in range(2):
                    nc.vector.tensor_copy(
                        vt[h][:, half * 2048 + q * 512:
                              half * 2048 + (q + 1) * 512],
                        pss[q][64 * h:64 * h + 64, :])

        def wo_block(b, h, nch):
            """Output rows [128h, 128h+128) of batch b, cols [512nch, +512)."""
            ps = ps_w.tile([128, 512], F32, tag="wo", name=f"psw{b}_{h}_{nch}")
            for m in range(16):
                lhs = vt[h][0:64, b * T + m: b * T + m + 16 * 127 + 1:16]
                nc.tensor.matmul(ps[:], lhs,
                                 wo_sb[:, m, nch * 512:(nch + 1) * 512],
                                 start=(m == 0), stop=(m == 15))
            ob = outp.tile([128, 512], F32, tag="ob", name=f"ob{b}_{h}_{nch}")
            nc.vector.tensor_copy(ob[:], ps[:])
            nc.sync.dma_start(
                out_d[b, 128 * h:128 * h + 128,
                      nch * 512:(nch + 1) * 512], ob[:])

        proj_half(0)
        for h in range(2):
            for nch in range(2):
                wo_block(0, h, nch)
        proj_half(1)
        for h in range(2):
            for nch in range(2):
                wo_block(1, h, nch)


def _get_module():
    global _CACHED
    if _CACHED is None:
        _CACHED = _build_module()
    return _CACHED


def _round_f32r(a):
    """Round fp32 to the fp32r grid (RNE at 11 mantissa bits) — verified
    bit-identical to the hardware fp32->fp32r cast."""
    b = np.ascontiguousarray(a, np.float32).view(np.uint32).astype(np.uint64)
    lsb = (b >> 12) & 1
    out = (b + 0x7FF + lsb) & np.uint64(0xFFFFF000)
    return out.astype(np.uint32).view(np.float32)


def kernel(x, mask, Wq, Wk, Wv, Wo):
    global LAST_RESULTS
    x = np.asarray(x, dtype=np.float32)
    Wv = np.asarray(Wv, dtype=np.float32)
    Wo = np.asarray(Wo, dtype=np.float32)

    b, t, d = x.shape
    assert (b, t, d) == (B, T, D), (b, t, d)

    xT = _round_f32r(x.transpose(2, 0, 1).reshape(D, GT))
    # woT[di, m, n] = Wo.T[64m+di, n]
    woT = _round_f32r(Wo.T.reshape(16, 64, D).transpose(1, 0, 2))
    wv_r = _round_f32r(Wv)

    in_maps = []
    for c in range(NCORES):
        in_maps.append({
            "xT": xT,
            "woT": woT,
            "wv": np.ascontiguousarray(wv_r[:, 128 * c:128 * c + 128]),
        })

    nc = _get_module()
    res = run_bass_kernel_spmd(nc, in_maps, list(range(NCORES)))
    LAST_RESULTS = res
    out = np.concatenate([res.results[c]["out"] for c in range(NCORES)],
                         axis=1)
    return np.ascontiguousarray(out.astype(np.float32))


# revision 14
# speedup vs baseline: 1.1590x; 1.1590x over previous
"""nn_MultiHeadAttention_59253368815813 on 8 TRN2 NeuronCores.

The reference module is bug-faithful to its original nn.Module in two ways
that together collapse the computation:

  1. ``o = jnp.einsum('bhtl,bthd->bhtd', A, v)`` indexes ``v`` by the QUERY
     position ``t``, not the key position ``l``. ``l`` therefore only sums
     over the softmax weights, which sum to exactly 1 per row:
     ``o[b,h,t,d] == v[b,t,h,d]``. Q, K, the mask and the softmax never
     influence the output (verified vs the reference to 4e-7 rel).
  2. ``o.reshape(b, T, d)`` with no transpose scrambles (head, token) so the
     reshaped activation row tj = 128*h + s is the concatenation over
     m=0..15 of v[b, 16*s+m, h, :].

So the exact computation is  out = scramble(x @ Wv) @ Wo.T,  and the
scramble makes output rows depend on one head only. Sharding: core c owns
heads {2c, 2c+1}, i.e. Wv columns [128c, 128c+128) and output rows
[256c, 256c+256) of each batch; the host concatenates the row slabs.
No cross-core reduction needed.

Per core (fp32r matmuls, fp32 PSUM):
  vT[128ch, b*T+t] = Wv_slice^T @ x^T          (x^T fed from host)
  out[128h + s, n] = sum_m sum_di vT[64h+di, 16s+m] * Wo.T[64m+di, n]
    -> 16 accumulating matmuls whose lhsT is a stride-16 slice of vT.
"""

import sys
import types

import numpy as np

_TRN_REPO = "/opt/trn_rl_repo"
if _TRN_REPO not in sys.path:
    sys.path.insert(0, _TRN_REPO)


def _install_ntff_shim():
    """antenv.axon_hooks is absent in this container; provide it so
    BASS_TRACE=1 profiling works. No-op if the real module exists."""
    try:
        import antenv  # noqa: F401
    except ImportError:
        return
    if "antenv.axon_hooks" in sys.modules:
        return
    try:
        import antenv.axon_hooks  # noqa: F401
        return
    except ImportError:
        pass
    m = types.ModuleType("antenv.axon_hooks")
    m._hook = None
    m.set_axon_ntff_profile_hook = lambda h: setattr(m, "_hook", h)
    m.get_axon_ntff_profile_hook = lambda: m._hook
    sys.modules["antenv.axon_hooks"] = m
    try:
        from trn_agent_boot.trn_boot import _ntff_profile_via_ctypes

        hook = _ntff_profile_via_ctypes("/opt/axon/libaxon_pjrt.so")
        if hook is not None:
            m.set_axon_ntff_profile_hook(hook)
    except Exception:
        pass


_install_ntff_shim()

import concourse.mybir as mybir  # noqa: E402
import concourse.tile as tile  # noqa: E402
from concourse import bacc  # noqa: E402
from concourse.bass_utils import run_bass_kernel_spmd  # noqa: E402

F32 = mybir.dt.float32
F32R = mybir.dt.float32r

B = 2
T = 2048
D = 1024
NCORES = 8
GT = B * T          # 4096
NG = GT // 512      # 8 global 512-token chunks
NDCH = D // 128     # 8 contraction chunks for the projection

_CACHED = None
LAST_RESULTS = None


def _build_module():
    nc = bacc.Bacc("TRN2", target_bir_lowering=False, debug=False,
                   num_devices=NCORES)

    xT_d = nc.dram_tensor("xT", [D, GT], F32R, kind="ExternalInput").ap()
    wv_d = nc.dram_tensor("wv", [D, 128], F32R, kind="ExternalInput").ap()
    wo_d = nc.dram_tensor("woT", [64, 16, D], F32R, kind="ExternalInput").ap()
    out_d = nc.dram_tensor("out", [B, 256, D], F32, kind="ExternalOutput").ap()

    with tile.TileContext(nc) as tc:
        _emit(nc, tc, xT_d, wv_d, wo_d, out_d)
    nc.compile()
    return nc


def _emit(nc, tc, xT_d, wv_d, wo_d, out_d):
    from contextlib import ExitStack

    ctx = ExitStack()
    with ctx:
        wpool = ctx.enter_context(tc.tile_pool(name="w", bufs=1))
        xtp = ctx.enter_context(tc.tile_pool(name="xt", bufs=3))
        vtp = ctx.enter_context(tc.tile_pool(name="vt", bufs=1))
        outp = ctx.enter_context(tc.tile_pool(name="outsb", bufs=3))
        ps_p = ctx.enter_context(tc.tile_pool(name="ps_p", bufs=4, space="PSUM"))
        ps_w = ctx.enter_context(tc.tile_pool(name="ps_w", bufs=2, space="PSUM"))

        # weights ride the ACT HWDGE ring; activations the SP ring (parallel)
        wv_sb = wpool.tile([128, NDCH, 128], F32R, tag="wv")
        nc.scalar.dma_start(wv_sb[:],
                            wv_d.rearrange("(ko ki) m -> ki ko m", ki=128))
        wo_sb = wpool.tile([64, 16, D], F32R, tag="wo")
        nc.scalar.dma_start(wo_sb[:], wo_d)

        # vT split by head so both heads' Wo matmuls read base-partition 0
        vt = [vtp.tile([64, GT], F32R, tag=f"vt{h}", name=f"vt{h}")
              for h in range(2)]

        def proj_half(half, after_j=None):
            """v^T for one 2048-token half (= one batch). after_j(j) lets the
            caller interleave other PE work between the 2MB-chunk groups."""
            pss = [ps_p.tile([128, 512], F32, tag="proj",
                             name=f"psp{half}_{q}") for q in range(4)]
            for j in range(4):
                xt = xtp.tile([128, 2, 2048], F32R, tag="xt",
                              name=f"xt{half}_{j}")
                nc.sync.dma_start(
                    xt[:], xT_d[j * 256:(j + 1) * 256,
                                half * 2048:(half + 1) * 2048]
                    .rearrange("(ko ki) t -> ki ko t", ki=128))
                for kk in range(2):
                    dch = 2 * j + kk
                    for q in range(4):
                        nc.tensor.matmul(pss[q][:], wv_sb[:, dch, :],
                                         xt[:, kk, q * 512:(q + 1) * 512],
                                         start=(dch == 0),
                                         stop=(dch == NDCH - 1))
                if after_j is not None:
                    after_j(j)
            for q in range(4):
                for h in range(2):
                    nc.vector.tensor_copy(
                        vt[h][:, half * 2048 + q * 512:
                              half * 2048 + (q + 1) * 512],
                        pss[q][64 * h:64 * h + 64, :])

        def wo_block(b, h, nch):
            """Output rows [128h, 128h+128) of batch b, cols [512nch, +512)."""
            ps = ps_w.tile([128, 512], F32, tag="wo", name=f"psw{b}_{h}_{nch}")
            for m in range(16):
                lhs = vt[h][0:64, b * T + m: b * T + m + 16 * 127 + 1:16]
                nc.tensor.matmul(ps[:], lhs,
                                 wo_sb[:, m, nch * 512:(nch + 1) * 512],
                                 start=(m == 0), stop=(m == 15))
            ob = outp.tile([128, 512], F32, tag="ob", name=f"ob{b}_{h}_{nch}")
            nc.vector.tensor_copy(ob[:], ps[:])
            nc.scalar.dma_start(
                out_d[b, 128 * h:128 * h + 128,
                      nch * 512:(nch + 1) * 512], ob[:])

        proj_half(0)
        # during half-1's DMA stream, fill PE gaps with batch-0 out-proj
        proj_half(1, after_j=lambda j: wo_block(0, j // 2, j % 2))
        for h in range(2):
            for nch in range(2):
                wo_block(1, h, nch)


def _get_module():
    global _CACHED
    if _CACHED is None:
        _CACHED = _build_module()
    return _CACHED


def _round_f32r(a):
    """Round fp32 to the fp32r grid (RNE at 11 mantissa bits) — verified
    bit-identical to the hardware fp32->fp32r cast."""
    b = np.ascontiguousarray(a, np.float32).view(np.uint32).astype(np.uint64)
    lsb = (b >> 12) & 1
    out = (b + 0x7FF + lsb) & np.uint64(0xFFFFF000)
    return out.astype(np.uint32).view(np.float32)


def kernel(x, mask, Wq, Wk, Wv, Wo):
    global LAST_RESULTS
    x = np.asarray(x, dtype=np.float32)
    Wv = np.asarray(Wv, dtype=np.float32)
    Wo = np.asarray(Wo, dtype=np.float32)

    b, t, d = x.shape
    assert (b, t, d) == (B, T, D), (b, t, d)

    xT = _round_f32r(x.transpose(2, 0, 1).reshape(D, GT))
    # woT[di, m, n] = Wo.T[64m+di, n]
    woT = _round_f32r(Wo.T.reshape(16, 64, D).transpose(1, 0, 2))
    wv_r = _round_f32r(Wv)

    in_maps = []
    for c in range(NCORES):
        in_maps.append({
            "xT": xT,
            "woT": woT,
            "wv": np.ascontiguousarray(wv_r[:, 128 * c:128 * c + 128]),
        })

    nc = _get_module()
    res = run_bass_kernel_spmd(nc, in_maps, list(range(NCORES)))
    LAST_RESULTS = res
    out = np.concatenate([res.results[c]["out"] for c in range(NCORES)],
                         axis=1)
    return np.ascontiguousarray(out.astype(np.float32))


# revision 15
# speedup vs baseline: 1.2231x; 1.0553x over previous
"""nn_MultiHeadAttention_59253368815813 on 8 TRN2 NeuronCores.

The reference module is bug-faithful to its original nn.Module in two ways
that together collapse the computation:

  1. ``o = jnp.einsum('bhtl,bthd->bhtd', A, v)`` indexes ``v`` by the QUERY
     position ``t``, not the key position ``l``. ``l`` therefore only sums
     over the softmax weights, which sum to exactly 1 per row:
     ``o[b,h,t,d] == v[b,t,h,d]``. Q, K, the mask and the softmax never
     influence the output (verified vs the reference to 4e-7 rel).
  2. ``o.reshape(b, T, d)`` with no transpose scrambles (head, token) so the
     reshaped activation row tj = 128*h + s is the concatenation over
     m=0..15 of v[b, 16*s+m, h, :].

So the exact computation is  out = scramble(x @ Wv) @ Wo.T,  and the
scramble makes output rows depend on one head only. Sharding: core c owns
heads {2c, 2c+1}, i.e. Wv columns [128c, 128c+128) and output rows
[256c, 256c+256) of each batch; the host concatenates the row slabs.
No cross-core reduction needed.

Per core (fp32r matmuls, fp32 PSUM):
  vT[128ch, b*T+t] = Wv_slice^T @ x^T          (x^T fed from host)
  out[128h + s, n] = sum_m sum_di vT[64h+di, 16s+m] * Wo.T[64m+di, n]
    -> 16 accumulating matmuls whose lhsT is a stride-16 slice of vT.
"""

import sys
import types

import numpy as np

_TRN_REPO = "/opt/trn_rl_repo"
if _TRN_REPO not in sys.path:
    sys.path.insert(0, _TRN_REPO)


def _install_ntff_shim():
    """antenv.axon_hooks is absent in this container; provide it so
    BASS_TRACE=1 profiling works. No-op if the real module exists."""
    try:
        import antenv  # noqa: F401
    except ImportError:
        return
    if "antenv.axon_hooks" in sys.modules:
        return
    try:
        import antenv.axon_hooks  # noqa: F401
        return
    except ImportError:
        pass
    m = types.ModuleType("antenv.axon_hooks")
    m._hook = None
    m.set_axon_ntff_profile_hook = lambda h: setattr(m, "_hook", h)
    m.get_axon_ntff_profile_hook = lambda: m._hook
    sys.modules["antenv.axon_hooks"] = m
    try:
        from trn_agent_boot.trn_boot import _ntff_profile_via_ctypes

        hook = _ntff_profile_via_ctypes("/opt/axon/libaxon_pjrt.so")
        if hook is not None:
            m.set_axon_ntff_profile_hook(hook)
    except Exception:
        pass


_install_ntff_shim()

import concourse.mybir as mybir  # noqa: E402
import concourse.tile as tile  # noqa: E402
from concourse import bacc  # noqa: E402
from concourse.bass_utils import run_bass_kernel_spmd  # noqa: E402

F32 = mybir.dt.float32
F32R = mybir.dt.float32r

B = 2
T = 2048
D = 1024
NCORES = 8
GT = B * T          # 4096
NG = GT // 512      # 8 global 512-token chunks
NDCH = D // 128     # 8 contraction chunks for the projection

_CACHED = None
LAST_RESULTS = None


def _build_module():
    nc = bacc.Bacc("TRN2", target_bir_lowering=False, debug=False,
                   num_devices=NCORES)

    xT_d = nc.dram_tensor("xT", [D, GT], F32R, kind="ExternalInput").ap()
    wv_d = nc.dram_tensor("wv", [128, NDCH, 128], F32R,
                          kind="ExternalInput").ap()
    wo_d = nc.dram_tensor("woT", [64, 16, D], F32R, kind="ExternalInput").ap()
    out_d = nc.dram_tensor("out", [B, 256, D], F32, kind="ExternalOutput").ap()

    with tile.TileContext(nc) as tc:
        _emit(nc, tc, xT_d, wv_d, wo_d, out_d)
    nc.compile()
    return nc


def _emit(nc, tc, xT_d, wv_d, wo_d, out_d):
    from contextlib import ExitStack

    ctx = ExitStack()
    with ctx:
        wpool = ctx.enter_context(tc.tile_pool(name="w", bufs=1))
        xtp = ctx.enter_context(tc.tile_pool(name="xt", bufs=3))
        vtp = ctx.enter_context(tc.tile_pool(name="vt", bufs=1))
        outp = ctx.enter_context(tc.tile_pool(name="outsb", bufs=3))
        ps_p = ctx.enter_context(tc.tile_pool(name="ps_p", bufs=4, space="PSUM"))
        ps_w = ctx.enter_context(tc.tile_pool(name="ps_w", bufs=2, space="PSUM"))

        # weights ride the ACT HWDGE ring; activations the SP ring (parallel)
        wv_sb = wpool.tile([128, NDCH, 128], F32R, tag="wv")
        nc.scalar.dma_start(wv_sb[:], wv_d)
        wo_sb = wpool.tile([64, 16, D], F32R, tag="wo")
        nc.scalar.dma_start(wo_sb[:], wo_d)

        # vT split by head so both heads' Wo matmuls read base-partition 0
        vt = [vtp.tile([64, GT], F32R, tag=f"vt{h}", name=f"vt{h}")
              for h in range(2)]

        def proj_half(half, after_j=None):
            """v^T for one 2048-token half (= one batch). after_j(j) lets the
            caller interleave other PE work between the 2MB-chunk groups."""
            pss = [ps_p.tile([128, 512], F32, tag="proj",
                             name=f"psp{half}_{q}") for q in range(4)]
            for j in range(4):
                xt = xtp.tile([128, 2, 2048], F32R, tag="xt",
                              name=f"xt{half}_{j}")
                nc.sync.dma_start(
                    xt[:], xT_d[j * 256:(j + 1) * 256,
                                half * 2048:(half + 1) * 2048]
                    .rearrange("(ko ki) t -> ki ko t", ki=128))
                for kk in range(2):
                    dch = 2 * j + kk
                    for q in range(4):
                        nc.tensor.matmul(pss[q][:], wv_sb[:, dch, :],
                                         xt[:, kk, q * 512:(q + 1) * 512],
                                         start=(dch == 0),
                                         stop=(dch == NDCH - 1))
                if after_j is not None:
                    after_j(j)
            for q in range(4):
                for h in range(2):
                    nc.vector.tensor_copy(
                        vt[h][:, half * 2048 + q * 512:
                              half * 2048 + (q + 1) * 512],
                        pss[q][64 * h:64 * h + 64, :])

        def wo_block(b, h, nch):
            """Output rows [128h, 128h+128) of batch b, cols [512nch, +512)."""
            ps = ps_w.tile([128, 512], F32, tag="wo", name=f"psw{b}_{h}_{nch}")
            for m in range(16):
                lhs = vt[h][0:64, b * T + m: b * T + m + 16 * 127 + 1:16]
                nc.tensor.matmul(ps[:], lhs,
                                 wo_sb[:, m, nch * 512:(nch + 1) * 512],
                                 start=(m == 0), stop=(m == 15))
            ob = outp.tile([128, 512], F32, tag="ob", name=f"ob{b}_{h}_{nch}")
            nc.vector.tensor_copy(ob[:], ps[:])
            nc.scalar.dma_start(
                out_d[b, 128 * h:128 * h + 128,
                      nch * 512:(nch + 1) * 512], ob[:])

        proj_half(0)
        # during half-1's DMA stream, fill PE gaps with batch-0 out-proj
        proj_half(1, after_j=lambda j: wo_block(0, j // 2, j % 2))
        for h in range(2):
            for nch in range(2):
                wo_block(1, h, nch)


def _get_module():
    global _CACHED
    if _CACHED is None:
        _CACHED = _build_module()
    return _CACHED


def _round_f32r(a):
    """Round fp32 to the fp32r grid (RNE at 11 mantissa bits) — verified
    bit-identical to the hardware fp32->fp32r cast."""
    b = np.ascontiguousarray(a, np.float32).view(np.uint32).astype(np.uint64)
    lsb = (b >> 12) & 1
    out = (b + 0x7FF + lsb) & np.uint64(0xFFFFF000)
    return out.astype(np.uint32).view(np.float32)


def kernel(x, mask, Wq, Wk, Wv, Wo):
    global LAST_RESULTS
    x = np.asarray(x, dtype=np.float32)
    Wv = np.asarray(Wv, dtype=np.float32)
    Wo = np.asarray(Wo, dtype=np.float32)

    b, t, d = x.shape
    assert (b, t, d) == (B, T, D), (b, t, d)

    xT = _round_f32r(x.transpose(2, 0, 1).reshape(D, GT))
    # woT[di, m, n] = Wo.T[64m+di, n]
    woT = _round_f32r(Wo.T.reshape(16, 64, D).transpose(1, 0, 2))
    wv_r = _round_f32r(Wv)

    in_maps = []
    for c in range(NCORES):
        wv_c = wv_r[:, 128 * c:128 * c + 128]  # [1024, 128]
        wv_c = np.ascontiguousarray(
            wv_c.reshape(NDCH, 128, 128).transpose(1, 0, 2))
        in_maps.append({
            "xT": xT,
            "woT": woT,
            "wv": wv_c,
        })

    nc = _get_module()
    res = run_bass_kernel_spmd(nc, in_maps, list(range(NCORES)))
    LAST_RESULTS = res
    out = np.concatenate([res.results[c]["out"] for c in range(NCORES)],
                         axis=1)
    return np.ascontiguousarray(out.astype(np.float32))


# revision 16
# speedup vs baseline: 1.5920x; 1.3016x over previous
"""nn_MultiHeadAttention_59253368815813 on 8 TRN2 NeuronCores.

The reference module is bug-faithful to its original nn.Module in two ways
that together collapse the computation:

  1. ``o = jnp.einsum('bhtl,bthd->bhtd', A, v)`` indexes ``v`` by the QUERY
     position ``t``, not the key position ``l``. ``l`` therefore only sums
     over the softmax weights, which sum to exactly 1 per row:
     ``o[b,h,t,d] == v[b,t,h,d]``. Q, K, the mask and the softmax never
     influence the output (verified vs the reference to 4e-7 rel).
  2. ``o.reshape(b, T, d)`` with no transpose scrambles (head, token) so the
     reshaped activation row tj = 128*h + s is the concatenation over
     m=0..15 of v[b, 16*s+m, h, :].

So the exact computation is  out = scramble(x @ Wv) @ Wo.T,  and the
scramble makes output rows depend on one head only. Sharding: core c owns
heads {2c, 2c+1}, i.e. Wv columns [128c, 128c+128) and output rows
[256c, 256c+256) of each batch; the host concatenates the row slabs.
No cross-core reduction needed.

Per core (fp32r matmuls, fp32 PSUM):
  vT[128ch, u] = Wv_slice^T @ x^T   where the host feeds x^T with tokens
  permuted to u = m*128 + r (t = 16r + m), so the reshape scramble becomes
  contiguous: the PSUM evacuation writes vt2[64*(m%2)+di, (m//2)*128+r] and
  the output projection is 8 accumulating K=128 matmuls per output tile:
  out[128h + r, n] = sum_m2 vt2_chunk(m2)^T @ WoT[128*m2:128*m2+128, n].
"""

import sys
import types

import numpy as np

_TRN_REPO = "/opt/trn_rl_repo"
if _TRN_REPO not in sys.path:
    sys.path.insert(0, _TRN_REPO)


def _install_ntff_shim():
    """antenv.axon_hooks is absent in this container; provide it so
    BASS_TRACE=1 profiling works. No-op if the real module exists."""
    try:
        import antenv  # noqa: F401
    except ImportError:
        return
    if "antenv.axon_hooks" in sys.modules:
        return
    try:
        import antenv.axon_hooks  # noqa: F401
        return
    except ImportError:
        pass
    m = types.ModuleType("antenv.axon_hooks")
    m._hook = None
    m.set_axon_ntff_profile_hook = lambda h: setattr(m, "_hook", h)
    m.get_axon_ntff_profile_hook = lambda: m._hook
    sys.modules["antenv.axon_hooks"] = m
    try:
        from trn_agent_boot.trn_boot import _ntff_profile_via_ctypes

        hook = _ntff_profile_via_ctypes("/opt/axon/libaxon_pjrt.so")
        if hook is not None:
            m.set_axon_ntff_profile_hook(hook)
    except Exception:
        pass


_install_ntff_shim()

import concourse.mybir as mybir  # noqa: E402
import concourse.tile as tile  # noqa: E402
from concourse import bacc  # noqa: E402
from concourse.bass_utils import run_bass_kernel_spmd  # noqa: E402

F32 = mybir.dt.float32
F32R = mybir.dt.float32r

B = 2
T = 2048
D = 1024
NCORES = 8
GT = B * T          # 4096
NG = GT // 512      # 8 global 512-token chunks
NDCH = D // 128     # 8 contraction chunks for the projection

_CACHED = None
LAST_RESULTS = None


def _build_module():
    nc = bacc.Bacc("TRN2", target_bir_lowering=False, debug=False,
                   num_devices=NCORES)

    xT_d = nc.dram_tensor("xT", [D, GT], F32R, kind="ExternalInput").ap()
    wv_d = nc.dram_tensor("wv", [128, NDCH, 128], F32R,
                          kind="ExternalInput").ap()
    wo_d = nc.dram_tensor("woT", [128, 8, D], F32R,
                          kind="ExternalInput").ap()
    out_d = nc.dram_tensor("out", [B, 256, D], F32, kind="ExternalOutput").ap()

    with tile.TileContext(nc) as tc:
        _emit(nc, tc, xT_d, wv_d, wo_d, out_d)
    nc.compile()
    return nc


def _emit(nc, tc, xT_d, wv_d, wo_d, out_d):
    from contextlib import ExitStack

    ctx = ExitStack()
    with ctx:
        wpool = ctx.enter_context(tc.tile_pool(name="w", bufs=1))
        xtp = ctx.enter_context(tc.tile_pool(name="xt", bufs=3))
        vtp = ctx.enter_context(tc.tile_pool(name="vt", bufs=1))
        outp = ctx.enter_context(tc.tile_pool(name="outsb", bufs=3))
        ps_p = ctx.enter_context(tc.tile_pool(name="ps_p", bufs=4, space="PSUM"))
        ps_w = ctx.enter_context(tc.tile_pool(name="ps_w", bufs=2, space="PSUM"))

        # weights ride the ACT HWDGE ring; activations the SP ring (parallel)
        wv_sb = wpool.tile([128, NDCH, 128], F32R, tag="wv")
        nc.scalar.dma_start(wv_sb[:], wv_d)
        wo_sb = wpool.tile([128, 8, D], F32R, tag="wo")
        nc.scalar.dma_start(wo_sb[:], wo_d)

        # vt2[h][64*(m%2)+di, b*1024 + (m//2)*128 + r] = v[b, t=16r+m, 64h+di]
        vt = [vtp.tile([128, GT // 2], F32R, tag=f"vt{h}", name=f"vt{h}")
              for h in range(2)]

        def proj_half(half, after_j=None):
            """v^T for one 2048-token half (= one batch). after_j(j) lets the
            caller interleave other PE work between the 2MB-chunk groups."""
            pss = [ps_p.tile([128, 512], F32, tag="proj",
                             name=f"psp{half}_{q}") for q in range(4)]
            for j in range(4):
                xt = xtp.tile([128, 2, 2048], F32R, tag="xt",
                              name=f"xt{half}_{j}")
                nc.sync.dma_start(
                    xt[:], xT_d[j * 256:(j + 1) * 256,
                                half * 2048:(half + 1) * 2048]
                    .rearrange("(ko ki) t -> ki ko t", ki=128))
                for kk in range(2):
                    dch = 2 * j + kk
                    for q in range(4):
                        nc.tensor.matmul(pss[q][:], wv_sb[:, dch, :],
                                         xt[:, kk, q * 512:(q + 1) * 512],
                                         start=(dch == 0),
                                         stop=(dch == NDCH - 1))
                if after_j is not None:
                    after_j(j)
            for q in range(4):
                for h in range(2):
                    for mm in range(4):
                        m = q * 4 + mm
                        j, m2 = m % 2, m // 2
                        nc.vector.tensor_copy(
                            vt[h][64 * j:64 * j + 64,
                                  half * 1024 + m2 * 128:
                                  half * 1024 + (m2 + 1) * 128],
                            pss[q][64 * h:64 * h + 64,
                                   mm * 128:(mm + 1) * 128])

        def wo_block(b, h, nch):
            """Output rows [128h, 128h+128) of batch b, cols [512nch, +512)."""
            ps = ps_w.tile([128, 512], F32, tag="wo", name=f"psw{b}_{h}_{nch}")
            for m2 in range(8):
                lhs = vt[h][:, b * 1024 + m2 * 128:b * 1024 + (m2 + 1) * 128]
                nc.tensor.matmul(ps[:], lhs,
                                 wo_sb[:, m2, nch * 512:(nch + 1) * 512],
                                 start=(m2 == 0), stop=(m2 == 7))
            ob = outp.tile([128, 512], F32, tag="ob", name=f"ob{b}_{h}_{nch}")
            nc.vector.tensor_copy(ob[:], ps[:])
            nc.scalar.dma_start(
                out_d[b, 128 * h:128 * h + 128,
                      nch * 512:(nch + 1) * 512], ob[:])

        proj_half(0)
        # during half-1's DMA stream, fill PE gaps with batch-0 out-proj
        proj_half(1, after_j=lambda j: wo_block(0, j // 2, j % 2))
        for h in range(2):
            for nch in range(2):
                wo_block(1, h, nch)


def _get_module():
    global _CACHED
    if _CACHED is None:
        _CACHED = _build_module()
    return _CACHED


def _round_f32r(a):
    """Round fp32 to the fp32r grid (RNE at 11 mantissa bits) — verified
    bit-identical to the hardware fp32->fp32r cast."""
    b = np.ascontiguousarray(a, np.float32).view(np.uint32).astype(np.uint64)
    lsb = (b >> 12) & 1
    out = (b + 0x7FF + lsb) & np.uint64(0xFFFFF000)
    return out.astype(np.uint32).view(np.float32)


def kernel(x, mask, Wq, Wk, Wv, Wo):
    global LAST_RESULTS
    x = np.asarray(x, dtype=np.float32)
    Wv = np.asarray(Wv, dtype=np.float32)
    Wo = np.asarray(Wo, dtype=np.float32)

    b, t, d = x.shape
    assert (b, t, d) == (B, T, D), (b, t, d)

    # x^T with tokens permuted to u = m*128 + r  (original t = 16r + m)
    xT = x.transpose(2, 0, 1).reshape(D, B, 128, 16)
    xT = _round_f32r(xT.swapaxes(2, 3).reshape(D, GT))
    # woT[p, m2, n] = Wo.T[128*m2 + p, n]
    woT = _round_f32r(Wo.T.reshape(8, 128, D).transpose(1, 0, 2))
    wv_r = _round_f32r(Wv)

    in_maps = []
    for c in range(NCORES):
        wv_c = wv_r[:, 128 * c:128 * c + 128]  # [1024, 128]
        wv_c = np.ascontiguousarray(
            wv_c.reshape(NDCH, 128, 128).transpose(1, 0, 2))
        in_maps.append({
            "xT": xT,
            "woT": woT,
            "wv": wv_c,
        })

    nc = _get_module()
    res = run_bass_kernel_spmd(nc, in_maps, list(range(NCORES)))
    LAST_RESULTS = res
    out = np.concatenate([res.results[c]["out"] for c in range(NCORES)],
                         axis=1)
    return np.ascontiguousarray(out.astype(np.float32))
